# revision 1
# baseline (speedup 1.0000x reference)
"""Trainium2 Bass kernel for DynamicPathCrossAttention.

Sharding: batch-parallel — core b computes batch element b end-to-end. The
path-gating MLP is evaluated on the host from the runtime inputs; each core
only computes cross-attention for its batch element's TOP_K=2 selected paths.

Weight folding (host, shared across cores): because the reference chain is
linear around the softmax, adjacent projection pairs collapse:
  logits = Q Wq^T Wk S^T          -> G_p = Wq^T @ Wk_p     (logits = Q G S^T)
  out    = attn S Wv^T Wo^T (...) -> H_p = Wo @ Wv_p       (out = attn S H^T)
so the device never materializes Qp, K, or V — 8 big matmul units per core
instead of 10. Bias algebra: the per-q logit terms cancel inside softmax; the
per-k term ships as an exp() bias column vb = (S @ Wk^T bq) / sqrt(D); bv
folds into an effective output bias boe = bo + sum_p w_p (Wo @ bv_p).

Device pipeline per path (all contractions on SBUF partitions, zero
on-device transposes; all matmuls float32r = full PE rate, ~1e-4 rel err):
  TMP[d',q]   = sum_d  G[d,d'] QT[d,q]        (lhsT=G resident, rhs=QT chunk)
  logitsT[k,q]= sum_d' ST[d',k] TMP[d',q]     (lhsT=ST resident, rhs=TMP)
  expT        = exp(logitsT/sqrt(D) + vb[k])  (ACT from PSUM, bias fused)
  rowsum[1,q] = sum_k expT[k,q]               (ones-matmul)
  AOS[d',q]   = sum_k SN[k,d'] expT[k,q]      (lhsT=SN chunk, rhs=expT)
  AOSs        = AOS * (w_p/rowsum broadcast)  (DVE from PSUM)
  outT[o,q]  += sum_d' HT[d',o] AOSs[d',q]    (lhsT=HT resident; path-0 half
                                               stashed in SBUF, path-1 adds)
"""

import numpy as np

D = 1024
P = 4
TOP_K = 2
B = 8
LQ = 1024
LK = 1024
N_CORES = 8

_CACHE = {}


def _build_program():
    import concourse.bass as bass  # noqa: F401
    import concourse.mybir as mybir
    import concourse.tile as tile
    from concourse import bacc

    f32 = mybir.dt.float32
    f32r = mybir.dt.float32r
    Exp = mybir.ActivationFunctionType.Exp
    Identity = mybir.ActivationFunctionType.Identity
    ADD = mybir.AluOpType.add
    MULT = mybir.AluOpType.mult

    nc = bacc.Bacc(
        "TRN2", target_bir_lowering=False, debug=False, enable_asserts=False
    )

    def din(name, shape):
        return nc.dram_tensor(name, shape, f32, kind="ExternalInput").ap()

    QT = din("QT", [D, LQ])
    ST_d = [din(f"S{p}T", [D, LK]) for p in range(2)]
    SN_d = [din(f"SN{p}", [LK, D]) for p in range(2)]
    G_d = [din(f"G{p}", [D, D]) for p in range(2)]
    HT_d = [din(f"HT{p}", [D, D]) for p in range(2)]
    vb_d = [din(f"vb{p}", [LK, 1]) for p in range(2)]
    boe_c = din("boe", [D, 1])
    wgt = din("wgt", [1, 2])
    ones_col_d = din("ones_col", [128, 1])
    ones_row_d = din("ones_row", [1, 128])
    outT = nc.dram_tensor("outT", [D, LQ], f32, kind="ExternalOutput").ap()

    SCALE = 1.0 / float(np.sqrt(D))
    nD = D // 128

    with tile.TileContext(nc) as tc:
        import contextlib

        with contextlib.ExitStack() as ctx:
            const = ctx.enter_context(tc.tile_pool(name="const", bufs=1))
            stream = ctx.enter_context(tc.tile_pool(name="stream", bufs=10))
            tap = ctx.enter_context(tc.tile_pool(name="tap", bufs=1))
            stp = ctx.enter_context(tc.tile_pool(name="stp", bufs=1))
            kvp = ctx.enter_context(tc.tile_pool(name="kvp", bufs=1))
            expp = ctx.enter_context(tc.tile_pool(name="expp", bufs=1))
            o0p = ctx.enter_context(tc.tile_pool(name="o0p", bufs=1))
            smallp = ctx.enter_context(tc.tile_pool(name="smallp", bufs=2))
            vecp = ctx.enter_context(tc.tile_pool(name="vecp", bufs=1))
            osbp = ctx.enter_context(tc.tile_pool(name="osbp", bufs=4))
            psp = ctx.enter_context(tc.tile_pool(name="psp", bufs=8, space="PSUM"))
            dramp = ctx.enter_context(tc.tile_pool(name="dramp", bufs=2, space="DRAM"))

            # ---- constants (DMAs deferred behind the first compute chunks) --
            ones_col = const.tile([128, 1], f32r)
            vb_t = [const.tile([128, nD], f32, name=f"vb_t{p}") for p in range(2)]
            boe_t = const.tile([128, nD], f32)
            wgt_sb = const.tile([1, 2], f32)

            def emit_const_dmas():
                nc.sync.dma_start(ones_col[:], ones_col_d[:].bitcast(f32r))
                for p in range(2):
                    nc.sync.dma_start(
                        vb_t[p][:], vb_d[p].rearrange("(t p) o -> p (t o)", p=128)
                    )
                nc.sync.dma_start(
                    boe_t[:], boe_c.rearrange("(t p) o -> p (t o)", p=128)
                )
                nc.sync.dma_start(wgt_sb[:], wgt[:])

            def load_st_tile(p, d_t):
                s_tile = stp.tile([128, LK], f32r, tag=f"st{d_t}", name=f"st{d_t}")
                nc.sync.dma_start(
                    s_tile[:],
                    ST_d[p][d_t * 128 : (d_t + 1) * 128, :].bitcast(f32r),
                )
                return s_tile

            out0 = [
                o0p.tile([128, LQ], f32, name=f"out0_{i}") for i in range(nD)
            ]

            for p in range(2):
                # =====================================================
                # TMP[d', q] = sum_d G[d, d'] QT[d, q]
                # G resident in kv slots; QT streams once per path.
                # ST for this path trickles in behind.
                # =====================================================
                g_res = []
                st = []
                tmp_t = [
                    tap.tile([128, LQ], f32r, tag=f"ta{i}", name=f"tmp{i}")
                    for i in range(nD)
                ]
                for q_b in range(2):
                    ps_t = [
                        psp.tile([128, 512], f32, tag="acc", name="ps_t")
                        for _ in range(8)
                    ]
                    for d_t in range(8):
                        qt_ch = stream.tile([128, 512], f32r, tag="wc", name="qtc")
                        nc.sync.dma_start(
                            qt_ch[:],
                            QT[
                                d_t * 128 : (d_t + 1) * 128,
                                q_b * 512 : (q_b + 1) * 512,
                            ].bitcast(f32r),
                        )
                        if q_b == 0:
                            if p == 0 and d_t == 0:
                                # two independently-waitable half tiles so the
                                # first matmuls start on the first 256KB
                                ga = kvp.tile([128, 512], f32r, tag="kv0a", name="g0a")
                                nc.sync.dma_start(
                                    ga[:], G_d[p][0:128, 0:512].bitcast(f32r)
                                )
                                gb = kvp.tile([128, 512], f32r, tag="kv0b", name="g0b")
                                nc.sync.dma_start(
                                    gb[:], G_d[p][0:128, 512:1024].bitcast(f32r)
                                )
                                g_res.append((ga, gb))
                            else:
                                g_tile = kvp.tile(
                                    [128, D], f32r, tag=f"kv{d_t}", name=f"g{d_t}"
                                )
                                nc.sync.dma_start(
                                    g_tile[:],
                                    G_d[p][
                                        d_t * 128 : (d_t + 1) * 128, :
                                    ].bitcast(f32r),
                                )
                                g_res.append(g_tile)
                        if p == 0 and q_b == 0 and d_t == 2:
                            emit_const_dmas()
                        for dp_t in range(8):
                            g = g_res[d_t]
                            if isinstance(g, tuple):
                                lhsT = (
                                    g[0][:, dp_t * 128 : (dp_t + 1) * 128]
                                    if dp_t < 4
                                    else g[1][:, (dp_t - 4) * 128 : (dp_t - 3) * 128]
                                )
                            else:
                                lhsT = g[:, dp_t * 128 : (dp_t + 1) * 128]
                            nc.tensor.matmul(
                                ps_t[dp_t][:],
                                lhsT,
                                qt_ch[:],
                                start=(d_t == 0),
                                stop=(d_t == 7),
                            )
                        # trickle this path's S^T behind the TMP chunks
                        if q_b == 1 and d_t in (0, 2, 4, 6):
                            st.append(load_st_tile(p, len(st)))
                    for dp_t in range(8):
                        dst = tmp_t[dp_t][:, q_b * 512 : (q_b + 1) * 512]
                        if dp_t % 2 == 0:
                            nc.scalar.activation(dst, ps_t[dp_t][:], Identity)
                        else:
                            nc.vector.tensor_copy(dst, ps_t[dp_t][:])
                while len(st) < 8:
                    st.append(load_st_tile(p, len(st)))

                # =====================================================
                # logits + exp + row-sums (both q blocks)
                # =====================================================
                expt = [
                    [
                        expp.tile([128, 512], f32r, tag=f"ex{q_b}_{k_t}", name="expt")
                        for k_t in range(8)
                    ]
                    for q_b in range(2)
                ]
                sbc = [None, None]

                def emit_logits_exp(q_b):
                    for k_t in range(8):
                        ps = psp.tile([128, 512], f32, tag="acc", name="ps_l")
                        for dp_t in range(8):
                            nc.tensor.matmul(
                                ps[:],
                                st[dp_t][:, k_t * 128 : (k_t + 1) * 128],
                                tmp_t[dp_t][:, q_b * 512 : (q_b + 1) * 512],
                                start=(dp_t == 0),
                                stop=(dp_t == 7),
                            )
                        nc.scalar.activation(
                            expt[q_b][k_t][:],
                            ps[:],
                            Exp,
                            bias=vb_t[p][:, k_t : k_t + 1],
                            scale=SCALE,
                        )

                def emit_rowsum(q_b):
                    ps_s = psp.tile([1, 512], f32, tag="acc", name="ps_s")
                    for k_t in range(8):
                        nc.tensor.matmul(
                            ps_s[:],
                            ones_col[:],
                            expt[q_b][k_t][:],
                            start=(k_t == 0),
                            stop=(k_t == 7),
                        )
                    return ps_s

                def emit_sbc(q_b, ps_s):
                    rs = vecp.tile([1, 512], f32, tag="rs", name="rs")
                    nc.vector.reciprocal(rs[:], ps_s[:])
                    s_row = vecp.tile([1, 512], f32, tag="srow", name="s_row")
                    nc.vector.tensor_scalar_mul(s_row[:], rs[:], wgt_sb[0:1, p : p + 1])
                    # broadcast across partitions via a DRAM bounce (the PE
                    # stays out of it; DRAM-source partition_broadcast works)
                    srow_d = dramp.tile([1, 512], f32, tag="srd", name="srow_d")
                    nc.sync.dma_start(srow_d[:], s_row[:])
                    sb_t = smallp.tile([128, 512], f32, tag="sbc", name="sb_t")
                    nc.sync.dma_start(sb_t[:], srow_d[0:1, :].partition_broadcast(128))
                    sbc[q_b] = sb_t

                emit_logits_exp(0)
                ps_s0 = emit_rowsum(0)
                emit_logits_exp(1)
                emit_sbc(0, ps_s0)
                ps_s1 = emit_rowsum(1)
                emit_sbc(1, ps_s1)

                # HT resident: reuse the (now dead) ST slots
                ht_res = []
                for dp_t in range(8):
                    h_tile = stp.tile(
                        [128, D], f32r, tag=f"st{dp_t}", name=f"ht{dp_t}"
                    )
                    nc.sync.dma_start(
                        h_tile[:],
                        HT_d[p][dp_t * 128 : (dp_t + 1) * 128, :].bitcast(f32r),
                    )
                    ht_res.append(h_tile)

                # =====================================================
                # AOS[d', q] = sum_k SN[k, d'] expT[k, q], then scale by
                # sbc = w_p / rowsum  (PSUM -> SBUF fused with the copy)
                # =====================================================
                aoss = [
                    tap.tile([128, LQ], f32r, tag=f"ta{i}", name=f"aoss{i}")
                    for i in range(nD)
                ]
                for dp_h in range(2):
                    ps_a = [
                        [
                            psp.tile([128, 512], f32, tag="acc", name="ps_a")
                            for _ in range(2)
                        ]
                        for _ in range(4)
                    ]
                    for k_t in range(8):
                        snc = stream.tile([128, 512], f32r, tag="wc", name="snc")
                        nc.sync.dma_start(
                            snc[:],
                            SN_d[p][
                                k_t * 128 : (k_t + 1) * 128,
                                dp_h * 512 : (dp_h + 1) * 512,
                            ].bitcast(f32r),
                        )
                        for dp_i in range(4):
                            for q_b in range(2):
                                nc.tensor.matmul(
                                    ps_a[dp_i][q_b][:],
                                    snc[:, dp_i * 128 : (dp_i + 1) * 128],
                                    expt[q_b][k_t][:],
                                    start=(k_t == 0),
                                    stop=(k_t == 7),
                                )
                    for dp_i in range(4):
                        dp_t = dp_h * 4 + dp_i
                        for q_b in range(2):
                            nc.vector.tensor_tensor(
                                aoss[dp_t][:, q_b * 512 : (q_b + 1) * 512],
                                ps_a[dp_i][q_b][:],
                                sbc[q_b][:],
                                MULT,
                            )

                # =====================================================
                # outT[o, q] += sum_d' HT[d', o] AOSs[d', q]
                # path 0 stashes into SBUF (with boe bias); path 1 adds
                # and writes out.  o_t-outer so copy+DMA pipelines.
                # =====================================================
                for q_b in range(2):
                    for o_t in range(8):
                        ps = psp.tile([128, 512], f32, tag="acc", name="ps_o")
                        for dp_t in range(8):
                            nc.tensor.matmul(
                                ps[:],
                                ht_res[dp_t][:, o_t * 128 : (o_t + 1) * 128],
                                aoss[dp_t][:, q_b * 512 : (q_b + 1) * 512],
                                start=(dp_t == 0),
                                stop=(dp_t == 7),
                            )
                        if p == 0:
                            dst = out0[o_t][:, q_b * 512 : (q_b + 1) * 512]
                            if o_t % 2 == 0:
                                nc.scalar.activation(
                                    dst, ps[:], Identity,
                                    bias=boe_t[:, o_t : o_t + 1],
                                )
                            else:
                                nc.vector.tensor_scalar_add(
                                    dst, ps[:], boe_t[:, o_t : o_t + 1]
                                )
                        else:
                            osb = osbp.tile([128, 512], f32, tag="osb", name="osb")
                            if o_t == 7 and q_b == 1:
                                # split the final tile so copy and DMA pipeline
                                for h in range(2):
                                    sl = slice(h * 256, (h + 1) * 256)
                                    nc.vector.tensor_tensor(
                                        osb[:, sl],
                                        ps[:, sl],
                                        out0[o_t][:, q_b * 512 + h * 256 : q_b * 512 + (h + 1) * 256],
                                        ADD,
                                    )
                                    nc.sync.dma_start(
                                        outT[
                                            o_t * 128 : (o_t + 1) * 128,
                                            q_b * 512 + h * 256 : q_b * 512 + (h + 1) * 256,
                                        ],
                                        osb[:, sl],
                                    )
                            else:
                                nc.vector.tensor_tensor(
                                    osb[:],
                                    ps[:],
                                    out0[o_t][:, q_b * 512 : (q_b + 1) * 512],
                                    ADD,
                                )
                                nc.sync.dma_start(
                                    outT[
                                        o_t * 128 : (o_t + 1) * 128,
                                        q_b * 512 : (q_b + 1) * 512,
                                    ],
                                    osb[:],
                                )

    nc.compile()
    return nc


def _get_program():
    if "nc" not in _CACHE:
        _CACHE["nc"] = _build_program()
    return _CACHE["nc"]


def _host_gating(Q, Wq, bq, Wm1, bm1, Wm2, bm2):
    """Replicates the reference path-score MLP + top-k sparse weights."""
    Qm = Q.astype(np.float64).mean(axis=1)  # [B, D]
    pooled = Qm @ Wq.astype(np.float64).T + bq.astype(np.float64)
    h = np.maximum(pooled @ Wm1.astype(np.float64).T + bm1.astype(np.float64), 0.0)
    pl = h @ Wm2.astype(np.float64).T + bm2.astype(np.float64)  # [B, P]
    pl = pl - pl.max(axis=1, keepdims=True)
    e = np.exp(pl)
    scores = e / e.sum(axis=1, keepdims=True)
    idx = np.argsort(-scores, axis=1, kind="stable")[:, :TOP_K]  # [B, 2]
    w = np.take_along_axis(scores, idx, axis=1)
    wn = w / (w.sum(axis=1, keepdims=True) + 1e-8)
    return idx.astype(np.int64), wn.astype(np.float32)


def kernel(**inputs):
    from concourse.bass_utils import run_bass_kernel_spmd

    Q = np.asarray(inputs["Q"], dtype=np.float32)
    src = np.asarray(inputs["src"], dtype=np.float32)
    Wq = np.asarray(inputs["Wq"], dtype=np.float32)
    bq = np.asarray(inputs["bq"], dtype=np.float32)
    Wk = np.asarray(inputs["Wk"], dtype=np.float32)
    bk = np.asarray(inputs["bk"], dtype=np.float32)  # noqa: F841  (cancels in softmax)
    Wv = np.asarray(inputs["Wv"], dtype=np.float32)
    bv = np.asarray(inputs["bv"], dtype=np.float32)
    Wm1 = np.asarray(inputs["Wm1"], dtype=np.float32)
    bm1 = np.asarray(inputs["bm1"], dtype=np.float32)
    Wm2 = np.asarray(inputs["Wm2"], dtype=np.float32)
    bm2 = np.asarray(inputs["bm2"], dtype=np.float32)
    Wo = np.asarray(inputs["Wo"], dtype=np.float32)
    bo = np.asarray(inputs["bo"], dtype=np.float32)

    idx, wn = _host_gating(Q, Wq, bq, Wm1, bm1, Wm2, bm2)
    SCALE = 1.0 / float(np.sqrt(D))

    nc = _get_program()

    # host-folded weights, shared across cores (<=4 selected paths)
    sel = sorted(set(idx.flatten().tolist()))
    WqT = Wq.T
    G = {p: np.ascontiguousarray(WqT @ Wk[p]) for p in sel}
    HT = {p: np.ascontiguousarray((Wo @ Wv[p]).T) for p in sel}
    g2 = {p: Wk[p].T @ bq for p in sel}
    Wobv = {p: Wo @ bv[p] for p in sel}
    ones_col = np.ones((128, 1), np.float32)
    ones_row = np.ones((1, 128), np.float32)

    in_maps = []
    for b in range(B):
        p0, p1 = int(idx[b, 0]), int(idx[b, 1])
        boe = bo + wn[b, 0] * Wobv[p0] + wn[b, 1] * Wobv[p1]
        m = {
            "QT": np.ascontiguousarray(Q[b].T),
            "S0T": np.ascontiguousarray(src[p0, b].T),
            "S1T": np.ascontiguousarray(src[p1, b].T),
            "SN0": np.ascontiguousarray(src[p0, b]),
            "SN1": np.ascontiguousarray(src[p1, b]),
            "G0": G[p0],
            "G1": G[p1],
            "HT0": HT[p0],
            "HT1": HT[p1],
            "vb0": np.ascontiguousarray(
                ((src[p0, b] @ g2[p0]) * SCALE).reshape(LK, 1).astype(np.float32)
            ),
            "vb1": np.ascontiguousarray(
                ((src[p1, b] @ g2[p1]) * SCALE).reshape(LK, 1).astype(np.float32)
            ),
            "boe": np.ascontiguousarray(boe.reshape(D, 1).astype(np.float32)),
            "wgt": np.ascontiguousarray(wn[b].reshape(1, 2)),
            "ones_col": ones_col,
            "ones_row": ones_row,
        }
        in_maps.append(m)

    res = run_bass_kernel_spmd(nc, in_maps, core_ids=list(range(N_CORES)))
    out = np.stack([res.results[b]["outT"].T for b in range(B)], axis=0)
    return np.ascontiguousarray(out).astype(np.float32)



# revision 61
# speedup vs baseline: 1.3388x; 1.3388x over previous
"""Trainium2 Bass kernel for DynamicPathCrossAttention.

Sharding: batch-parallel - core b computes batch element b end-to-end for its
TOP_K=2 gated paths (gating MLP evaluated on host, as it is a tiny [B,D]
computation). Weight folding on host (linear algebra around the softmax):
  G_p = Wq^T Wk_p   (logits = Q G_p S_p^T)
  H_p = Wo Wv_p     (out += w_p attn_p S_p H_p^T)

All four big [1024^3] contractions per path run as fp8e4m3 DoubleRow matmuls
(2 k-tiles per instruction, 0.5 cycles/row = 4x the fp32r rate) with 3-term
error compensation: every operand X is split host- or device-side into
X_hi = fp8(X), X_lo = fp8(X - X_hi), and the product is
  X@Y ~= X_hi@Y_hi + X_lo@Y_hi + X_hi@Y_lo     (lo@lo term dropped)
which restores ~bf16-level accuracy at 3/4 the fp32r PE time (vs 8 terms'
worth for full fp32 products). Measured end-to-end rel err ~2.6e-3.

Scale plumbing (all powers of two, host-chosen from cheap statistics):
  G' = G*sG so TMP = Q G' fits fp8 range; exp gets scale SCALE/sG on the ACT.
  A global logit shift ln_se (folded into the vb bias) guards exp overflow;
  it cancels in the softmax ratio. The gating weight ships premultiplied by
  s_a=2^9 so attention outputs land in fp8 range; H' = H^T*sH lifts H out of
  the fp8 subnormal range. The final output pass multiplies by 1/(sH*s_a)
  and adds the folded bias boe = bo + sum_p w_p Wo bv_p.

Per-path device pipeline (independent per 512-wide q-block):
  TMP[d',q]   = 3term(G', QT)        -> hi/lo split (ACT Identity + DVE sub)
  logitsT[k,q]= 3term(ST, TMP)       -> ACT Exp -> E16 (bf16) -> e_hi/e_lo
  rowsum[1,q] = ones DR matmul over e_hi+e_lo; sbc = w*s_a/rowsum broadcast
  AOS[d',q]   = 3term(SN, e)         -> *sbc (DVE) -> A16 -> a_hi/a_lo
  outT[o,q]   = 3term(H', A)         -> path0 stashes *inv+boe; path1 adds
"""

import numpy as np
import ml_dtypes

F8NP = ml_dtypes.float8_e4m3

D = 1024
P = 4
TOP_K = 2
B = 8
LQ = 1024
LK = 1024
N_CORES = 8
ND = D // 128  # 8 k-tiles of 128

_CACHE = {}


def _build_program(esc, inv):
    """esc = SCALE/sG (exp input scale); inv = 1/(sH*s_a) (output scale).
    The ones8 input arrives pre-filled with 1/s_a so the plain reciprocal
    of the rowsum directly yields sbc = s_a/rowsum."""
    import concourse.bass as bass  # noqa: F401
    import concourse.mybir as mybir
    import concourse.tile as tile
    from concourse import bacc

    f32 = mybir.dt.float32
    bf16 = mybir.dt.bfloat16
    f8 = mybir.dt.float8e4
    Exp = mybir.ActivationFunctionType.Exp
    Identity = mybir.ActivationFunctionType.Identity
    ADD = mybir.AluOpType.add
    SUB = mybir.AluOpType.subtract
    MULT = mybir.AluOpType.mult
    DR = mybir.MatmulPerfMode.DoubleRow

    SCALE = 1.0 / float(np.sqrt(D))

    nc = bacc.Bacc(
        "TRN2", target_bir_lowering=False, debug=False, enable_asserts=False
    )

    def din(name, shape, dt=f8):
        return nc.dram_tensor(name, shape, dt, kind="ExternalInput").ap()

    QT_d = [din(f"QT_{h}", [128, ND, LQ]) for h in range(2)]  # hi, lo
    G_d = [[din(f"G{j}_{h}", [128, ND, D]) for h in range(2)] for j in range(2)]
    ST_d = [[din(f"ST{j}_{h}", [128, ND, LK]) for h in range(2)] for j in range(2)]
    SN_d = [[din(f"SN{j}_{h}", [128, ND, D]) for h in range(2)] for j in range(2)]
    HT_d = [[din(f"HT{j}_{h}", [128, ND, D]) for h in range(2)] for j in range(2)]
    vb_d = [din(f"vb{j}", [128, ND], f32) for j in range(2)]
    boe_d = din("boe", [128, ND], f32)
    # dual-fp8 LDWEIGHTS needs a 16B-aligned k-tile stride in the weights
    # AP, so the ones live in a [*, ND, 16] tile (8 columns used)
    ones_d = din("ones8", [128, ND, 16])
    outT = nc.dram_tensor("outT", [D, LQ], f32, kind="ExternalOutput").ap()

    with tile.TileContext(nc) as tc:
        import contextlib

        with contextlib.ExitStack() as ctx:
            const = ctx.enter_context(tc.tile_pool(name="const", bufs=1))
            qtp = ctx.enter_context(tc.tile_pool(name="qtp", bufs=1))
            wtp = ctx.enter_context(tc.tile_pool(name="wtp", bufs=1))
            tmpp = ctx.enter_context(tc.tile_pool(name="tmpp", bufs=1))
            exp_p = ctx.enter_context(tc.tile_pool(name="exp_p", bufs=1))
            aop = ctx.enter_context(tc.tile_pool(name="aop", bufs=1))
            stashp = ctx.enter_context(tc.tile_pool(name="stashp", bufs=1))
            smallp = ctx.enter_context(tc.tile_pool(name="smallp", bufs=2))
            vecp = ctx.enter_context(tc.tile_pool(name="vecp", bufs=2))
            obufp = ctx.enter_context(tc.tile_pool(name="obufp", bufs=8))
            psp = ctx.enter_context(tc.tile_pool(name="psp", bufs=8, space="PSUM"))
            dramp = ctx.enter_context(tc.tile_pool(name="dramp", bufs=2, space="DRAM"))

            # ---------------- constants ----------------
            ones8 = const.tile([128, ND, 16], f8)
            vb_t = [const.tile([128, ND], f32, name=f"vb{j}") for j in range(2)]
            boe_t = const.tile([128, ND], f32)

            def emit_const_dmas():
                nc.sync.dma_start(ones8[:], ones_d[:])
                for j in range(2):
                    nc.sync.dma_start(vb_t[j][:], vb_d[j][:])
                nc.sync.dma_start(boe_t[:], boe_d[:])

            # ---------------- resident inputs ----------------
            qt = [qtp.tile([128, ND, LQ], f8, name=f"qt{h}") for h in range(2)]

            def load_wt(j, kind, dram_pair, own_slots=False):
                """Load a path stationary pair into tag-shared slots (or
                dedicated slots, avoiding the WAR wait on path0's reads)."""
                tiles = []
                for h in range(2):
                    tag = f"{kind}{h}{j}" if own_slots else f"{kind}{h}"
                    t = wtp.tile(
                        [128, ND, D], f8, tag=tag, name=f"{kind}{h}_{j}"
                    )
                    nc.sync.dma_start(t[:], dram_pair[h][:])
                    tiles.append(t)
                return tiles

            # First loads, chunked per k-pair in the exact order the first
            # (kp-major) T matmul group consumes them, so the PE starts after
            # ~1/16 of the bytes and stays fed at DMA rate.
            g = [
                wtp.tile([128, ND, D], f8, tag=f"g{h}", name=f"g{h}_0")
                for h in range(2)
            ]
            for kp in range(4):
                ks = slice(2 * kp, 2 * kp + 2)
                nc.sync.dma_start(g[0][:, ks, :], G_d[0][0][:, ks, :])
                nc.sync.dma_start(qt[0][:, ks, :], QT_d[0][:, ks, :])
                nc.sync.dma_start(g[1][:, ks, :], G_d[0][1][:, ks, :])
                nc.sync.dma_start(qt[1][:, ks, :], QT_d[1][:, ks, :])
            emit_const_dmas()
            st = load_wt(0, "st", ST_d[0])
            # sn0/ht0 are queued later (inside the schedule) so path-1's G
            # reaches the DMA queue before them

            # working tiles; tmp and a* double-buffered per q-block so
            # independent stages can interleave across q-blocks
            tmp = {
                qb: [
                    tmpp.tile([128, ND, 512], f8, tag=f"tmp{h}q{qb}",
                              name=f"tmp{h}q{qb}")
                    for h in range(2)
                ]
                for qb in range(2)
            }
            e16 = exp_p.tile([128, ND, 512], bf16, tag="e16", name="e16")
            eh = exp_p.tile([128, ND, 512], f8, tag="eh", name="eh")
            el = exp_p.tile([128, ND, 512], f8, tag="el", name="el")
            a16 = {
                qb: aop.tile([128, ND, 512], bf16, tag=f"a16q{qb}",
                             name=f"a16q{qb}")
                for qb in range(2)
            }
            ah = {
                qb: aop.tile([128, ND, 512], f8, tag=f"ahq{qb}", name=f"ahq{qb}")
                for qb in range(2)
            }
            al = {
                qb: aop.tile([128, ND, 512], f8, tag=f"alq{qb}", name=f"alq{qb}")
                for qb in range(2)
            }
            # path0's partial output staged in bf16 (halves SBUF; the ~0.4%
            # per-element rounding is far inside the error budget)
            stash = stashp.tile([128, ND, LQ], bf16, name="stash")

            def mm3(ps, lhs_pair, rhs_pair, lh_sl, rh_sl, term_major=False):
                """12 DR matmuls: 3 terms x 4 k-pairs accumulating into ps.

                kp-major order defers the k-tile-6/7 operands to the last
                instructions (hides trailing hi/lo extraction); term-major
                consumes all hi-parts first (hides the chunked initial DMA).
                """
                terms = (
                    (lhs_pair[0], rhs_pair[0]),
                    (lhs_pair[1], rhs_pair[0]),
                    (lhs_pair[0], rhs_pair[1]),
                )
                if term_major:
                    order = [(t, kp) for t in range(3) for kp in range(4)]
                else:
                    order = [(t, kp) for kp in range(4) for t in range(3)]
                for n, (t, kp) in enumerate(order):
                    ks = slice(2 * kp, 2 * kp + 2)
                    lh, rh = terms[t]
                    nc.tensor.matmul(
                        ps[:],
                        lh[:, ks, lh_sl],
                        rh[:, ks, rh_sl],
                        start=(n == 0),
                        stop=(n == 11),
                        perf_mode=DR,
                    )

            # PE warmup: ramp the tensor engine to full p-state during the
            # initial DMA window with throwaway matmuls on a zeroed tile.
            warm = const.tile([128, 2, 128], f8, name="warm")
            nc.vector.memset(warm[:], 0)
            ps_w = psp.tile([128, 512], f32, tag="acc", name="ps_w")
            for _ in range(45):
                nc.tensor.matmul(
                    ps_w[:, 0:128], warm[:], warm[:],
                    start=True, stop=True, perf_mode=DR,
                )

            wts = {0: dict(g=g, st=st), 1: {}}

            def emit_T(j, qb):
                w = wts[j]["g"]
                qsl = slice(qb * 512, (qb + 1) * 512)
                for dt in range(8):
                    ps = psp.tile([128, 512], f32, tag="acc", name="ps_t")
                    mm3(ps, w, qt, slice(dt * 128, (dt + 1) * 128), qsl)
                    nc.scalar.activation(tmp[qb][0][:, dt, :], ps[:], Identity)
                    nc.vector.tensor_tensor(
                        tmp[qb][1][:, dt, :], ps[:], tmp[qb][0][:, dt, :], SUB
                    )

            def emit_L(j, qb):
                w = wts[j]["st"]
                for kt in range(8):
                    ps = psp.tile([128, 512], f32, tag="acc", name="ps_l")
                    mm3(ps, w, tmp[qb], slice(kt * 128, (kt + 1) * 128),
                        slice(0, 512))
                    nc.scalar.activation(
                        e16[:, kt, :],
                        ps[:],
                        Exp,
                        bias=vb_t[j][:, kt : kt + 1],
                        scale=esc,
                    )
                    nc.scalar.activation(eh[:, kt, :], e16[:, kt, :], Identity)
                    eng = nc.vector if kt % 2 == 0 else nc.gpsimd
                    eng.tensor_tensor(
                        el[:, kt, :], e16[:, kt, :], eh[:, kt, :], SUB
                    )

            def emit_rowsum(j, qb):
                """rowsum over eh+el -> sbc = s_a / rowsum on all partitions.
                Uses only PE+ACT+Pool so the DVE queue (A16 extraction) never
                waits on work queued behind it."""
                ps_s = psp.tile([8, 512], f32, tag="acc", name="ps_s")
                n = 0
                for kp in range(4):
                    ks = slice(2 * kp, 2 * kp + 2)
                    for ex in (eh, el):
                        nc.tensor.matmul(
                            ps_s[:],
                            ones8[:, ks, 0:8],
                            ex[:, ks, :],
                            start=(n == 0),
                            stop=(n == 7),
                            perf_mode=DR,
                        )
                        n += 1
                s_row = vecp.tile([1, 512], f32, tag="srow", name="s_row")
                nc.vector.reciprocal(s_row[:], ps_s[0:1, :])
                sbc = smallp.tile([128, 512], f32, tag="sbc", name="sbc")
                nc.gpsimd.partition_broadcast(sbc[:], s_row[:])
                return sbc

            def emit_A(j, qb, rs_args, split_tail=False):
                """A matmul groups with the rowsum chain emitted after the
                first two groups (their extraction waits on sbc anyway)."""
                w = wts[j]["sn"]
                sbc = None

                def extract(dt):
                    if split_tail and dt >= 6:
                        # halve the 3-hop chain latency for the last tiles
                        # (their consumer stage starts right after this one)
                        for h in range(2):
                            hs = slice(h * 256, (h + 1) * 256)
                            nc.vector.tensor_tensor(
                                a16[qb][:, dt, hs], ps_t[dt][:, hs], sbc[:, hs],
                                MULT,
                            )
                            nc.scalar.activation(
                                ah[qb][:, dt, hs], a16[qb][:, dt, hs], Identity
                            )
                            eng = nc.vector if h == 0 else nc.gpsimd
                            eng.tensor_tensor(
                                al[qb][:, dt, hs], a16[qb][:, dt, hs],
                                ah[qb][:, dt, hs], SUB,
                            )
                        return
                    nc.vector.tensor_tensor(
                        a16[qb][:, dt, :], ps_t[dt][:], sbc[:], MULT
                    )
                    nc.scalar.activation(
                        ah[qb][:, dt, :], a16[qb][:, dt, :], Identity
                    )
                    eng = nc.vector if dt % 2 == 0 else nc.gpsimd
                    eng.tensor_tensor(
                        al[qb][:, dt, :], a16[qb][:, dt, :], ah[qb][:, dt, :],
                        SUB,
                    )

                ps_t = {}
                for dt in range(8):
                    ps_t[dt] = psp.tile([128, 512], f32, tag="acc", name="ps_a")
                    mm3(ps_t[dt], w, (eh, el), slice(dt * 128, (dt + 1) * 128),
                        slice(0, 512))
                    if dt == 1:
                        sbc = emit_rowsum(*rs_args)
                        extract(0)
                    if dt >= 1:
                        extract(dt)

            def emit_O(j, qb):
                w = wts[j]["ht"]
                qsl = slice(qb * 512, (qb + 1) * 512)
                for ot in range(8):
                    ps = psp.tile([128, 512], f32, tag="acc", name="ps_o")
                    mm3(ps, w, (ah[qb], al[qb]),
                        slice(ot * 128, (ot + 1) * 128), slice(0, 512))
                    if j == 0:
                        # alternate engines so extraction keeps pace
                        if ot % 2 == 0:
                            nc.scalar.activation(
                                stash[:, ot, qsl],
                                ps[:],
                                Identity,
                                bias=boe_t[:, ot : ot + 1],
                                scale=inv,
                            )
                        else:
                            nc.vector.tensor_scalar(
                                stash[:, ot, qsl],
                                ps[:],
                                inv,
                                boe_t[:, ot : ot + 1],
                                MULT,
                                ADD,
                            )
                    else:
                        ob = obufp.tile([128, 512], f32, tag="ob", name="ob")
                        # GPSIMD cannot read PSUM; DVE owns this extraction
                        nc.vector.scalar_tensor_tensor(
                            ob[:], ps[:], inv, stash[:, ot, qsl], MULT, ADD
                        )
                        nc.sync.dma_start(
                            outT[ot * 128 : (ot + 1) * 128, qsl], ob[:]
                        )

            # Interleaved schedule: each stage boundary's extraction trail is
            # covered by an independent stage's matmuls. Ordering constraints
            # (T(qb)->L(qb)->A(qb)->O(qb) per path, buffer reuse) are enforced
            # by emission order + tile semaphores. Path-1's G gets dedicated
            # slots so its early DMA needs no WAR wait; the other path-1
            # stationaries prefetch right after their slot's last reader.
            emit_T(0, 0)
            emit_T(0, 1)      # both T's first: L waits on the ST DMA anyway
            emit_L(0, 0)
            wts[1]["g"] = load_wt(1, "g", G_d[1], own_slots=True)
            emit_T(1, 0)      # covers L(0,0) extraction for A(0,0)
            wts[0]["sn"] = load_wt(0, "sn", SN_d[0])
            emit_A(0, 0, (0, 0))
            wts[0]["ht"] = load_wt(0, "ht", HT_d[0])
            emit_L(0, 1)
            wts[1]["st"] = load_wt(1, "st", ST_d[1])
            emit_O(0, 0)
            emit_A(0, 1, (0, 1))
            wts[1]["sn"] = load_wt(1, "sn", SN_d[1])
            emit_L(1, 0)      # covers path0 A(1) extraction
            emit_O(0, 1)
            wts[1]["ht"] = load_wt(1, "ht", HT_d[1])
            emit_T(1, 1)      # covers path0 O(1) extraction + out DMA
            emit_A(1, 0, (1, 0))
            emit_L(1, 1)
            emit_O(1, 0)      # covers L(1,1) extraction for A(1,1)
            emit_A(1, 1, (1, 1), split_tail=True)
            emit_O(1, 1)

    nc.compile()
    return nc


def _get_program(esc, inv):
    key = (esc, inv)
    if key not in _CACHE:
        _CACHE[key] = _build_program(esc, inv)
    return _CACHE[key]


def _host_gating(Q, Wq, bq, Wm1, bm1, Wm2, bm2):
    """Replicates the reference path-score MLP + top-k sparse weights."""
    Qm = Q.astype(np.float64).mean(axis=1)  # [B, D]
    pooled = Qm @ Wq.astype(np.float64).T + bq.astype(np.float64)
    h = np.maximum(pooled @ Wm1.astype(np.float64).T + bm1.astype(np.float64), 0.0)
    pl = h @ Wm2.astype(np.float64).T + bm2.astype(np.float64)  # [B, P]
    pl = pl - pl.max(axis=1, keepdims=True)
    e = np.exp(pl)
    scores = e / e.sum(axis=1, keepdims=True)
    idx = np.argsort(-scores, axis=1, kind="stable")[:, :TOP_K]  # [B, 2]
    w = np.take_along_axis(scores, idx, axis=1)
    wn = w / (w.sum(axis=1, keepdims=True) + 1e-8)
    return idx.astype(np.int64), wn.astype(np.float32)


def _pack(x):
    """[1024, N] contraction-major -> [128, 8, N] (partition, k-tile, free)."""
    return np.ascontiguousarray(x.reshape(ND, 128, -1).transpose(1, 0, 2))


def _split_pack(x):
    """fp8 hi/lo split then DR-pack both halves."""
    hi = x.astype(F8NP)
    lo = (x - hi.astype(np.float32)).astype(F8NP)
    return _pack(hi), _pack(lo)


def _pow2(x):
    return float(2.0 ** np.floor(np.log2(x)))


def kernel(**inputs):
    from concourse.bass_utils import run_bass_kernel_spmd

    Q = np.asarray(inputs["Q"], dtype=np.float32)
    src = np.asarray(inputs["src"], dtype=np.float32)
    Wq = np.asarray(inputs["Wq"], dtype=np.float32)
    bq = np.asarray(inputs["bq"], dtype=np.float32)
    Wk = np.asarray(inputs["Wk"], dtype=np.float32)
    bk = np.asarray(inputs["bk"], dtype=np.float32)  # noqa: F841 (cancels)
    Wv = np.asarray(inputs["Wv"], dtype=np.float32)
    bv = np.asarray(inputs["bv"], dtype=np.float32)
    Wm1 = np.asarray(inputs["Wm1"], dtype=np.float32)
    bm1 = np.asarray(inputs["bm1"], dtype=np.float32)
    Wm2 = np.asarray(inputs["Wm2"], dtype=np.float32)
    bm2 = np.asarray(inputs["bm2"], dtype=np.float32)
    Wo = np.asarray(inputs["Wo"], dtype=np.float32)
    bo = np.asarray(inputs["bo"], dtype=np.float32)

    idx, wn = _host_gating(Q, Wq, bq, Wm1, bm1, Wm2, bm2)
    SCALE = 1.0 / float(np.sqrt(D))

    sel = sorted(set(idx.flatten().tolist()))
    Gs = {p: Wq.T @ Wk[p] for p in sel}
    HTs = {p: (Wo @ Wv[p]).T for p in sel}
    g2v = {p: Wk[p].T @ bq for p in sel}
    Wobv = {p: Wo @ bv[p] for p in sel}
    vbs = {
        p: (src[p] @ g2v[p]) * SCALE if np.any(g2v[p])
        else np.zeros((B, LK), np.float32)
        for p in sel
    }

    # global power-of-two scales from cheap statistics
    sigQ = float(np.sqrt((Q**2).mean())) + 1e-30
    sigS = float(np.sqrt((src[sel] ** 2).mean())) + 1e-30
    sigT = max(
        float(np.sqrt((Gs[p] ** 2).mean() * D)) * sigQ for p in sel
    ) + 1e-30
    sG = _pow2(150.0 / (5.5 * sigT))
    sigH = max(float(np.sqrt((HTs[p] ** 2).mean())) for p in sel) + 1e-30
    sH = _pow2(2.0 / sigH)

    # exp overflow guard via a global logit shift folded into the vb bias
    # (a uniform shift of every logit cancels in the softmax ratio)
    sig_logit = sigT * sigS
    max_vb = max(float(np.abs(vbs[p]).max()) for p in sel)
    ln_se = min(0.0, float(np.log(150.0)) - (5.5 * sig_logit + max_vb))

    # attention outputs scaled by w*s_a must land in fp8 range; estimate
    # sqrt(sum attn^2) ~ e^{sig_l^2/2}/sqrt(LK) for gaussian logits
    sig_attn_out = sigS * float(np.exp(sig_logit**2 / 2)) / float(np.sqrt(LK))
    s_a = min(512.0, max(1.0, _pow2(24.0 / (5.5 * sig_attn_out))))
    inv = 1.0 / (sH * s_a)

    nc = _get_program(SCALE / sG, inv)

    Gp = {p: _split_pack(Gs[p] * sG) for p in sel}
    HTp = {p: _split_pack(HTs[p] * sH) for p in sel}
    # "ones" pre-scaled by 1/s_a (an exact power of two in fp8), so the
    # rowsum reciprocal directly yields sbc = s_a/rowsum
    ones8 = np.full((128, ND, 16), 1.0 / s_a, F8NP)

    in_maps = []
    for b in range(B):
        qh, ql = _split_pack(Q[b].T)
        m = {
            "QT_0": qh,
            "QT_1": ql,
            "ones8": ones8,
        }
        boe = bo.copy()
        for j in range(TOP_K):
            p = int(idx[b, j])
            S = src[p, b]
            sth, stl = _split_pack(np.ascontiguousarray(S.T))
            # gating weight folded into the SN operand (sbc is then just
            # s_a/rowsum, a compile-time-scaled reciprocal)
            snh, snl = _split_pack(S * wn[b, j])
            m[f"G{j}_0"], m[f"G{j}_1"] = Gp[p]
            m[f"ST{j}_0"], m[f"ST{j}_1"] = sth, stl
            m[f"SN{j}_0"], m[f"SN{j}_1"] = snh, snl
            m[f"HT{j}_0"], m[f"HT{j}_1"] = HTp[p]
            vb = vbs[p][b] + ln_se
            m[f"vb{j}"] = np.ascontiguousarray(
                vb.reshape(ND, 128).T.astype(np.float32)
            )
            boe = boe + wn[b, j] * Wobv[p]
        m["boe"] = np.ascontiguousarray(boe.reshape(ND, 128).T.astype(np.float32))
        in_maps.append(m)

    res = run_bass_kernel_spmd(nc, in_maps, core_ids=list(range(N_CORES)))
    out = np.stack([res.results[b]["outT"].T for b in range(B)], axis=0)
    return np.ascontiguousarray(out).astype(np.float32)
